# revision 2
# baseline (speedup 1.0000x reference)
"""Trainium2 Bass kernel for BestRQ vector-quantization codebook lookup.

Pipeline (per NeuronCore, data-parallel over batch):
  x (2048,512) --LayerNorm--> xn --PE transpose--> xnT (d-major)
  t^T = projW^T @ xn^T  (fp32 matmul, accumulated over d)
  t split into fp16 hi (th) + fp16 lo*2^11 (tl)
  codebook streamed in 512-column chunks, split into fp16 ch, ch*2^11 (chs),
  lo*2^11 (cls)
  score*2^11 = th@chs + th@cls + tl@ch   (3 fp16 passes, one PSUM, fp32 acc)
  s = score*2^11 - 2^11*0.5*||c||^2      (argmax invariant to the 2^11 scale)
  per-chunk argmax via DVE max8/max_index; global combine over 16 chunks.

Numerics: the fp16 hi/lo split covers 22 mantissa bits; measured max abs err
vs fp64 is ~2e-5 (fp32-parity) on the real data, so argmin labels match the
fp32 reference.
"""

import numpy as np

import concourse.bacc as bacc
import concourse.bass as bass
import concourse.mybir as mybir
import concourse.tile as tile
from concourse.bass_utils import run_bass_kernel_spmd
from concourse.masks import make_identity

B, L, D, H, C = 8, 2048, 512, 1024, 8192
LN_EPS = 1e-5
N_CORES = 8

TT = L // 128      # 16 token tiles
CCH = C // 512     # 16 codebook chunks
HT = H // 128      # 8 h tiles
DT = D // 128      # 4 d tiles
TOKC = L // 512    # 4 token chunks (projection)
SC = 2048.0        # 2^11 lo-part scale

F32 = mybir.dt.float32
F16 = mybir.dt.float16
I32 = mybir.dt.int32
U32 = mybir.dt.uint32


def build_nc(passes=3):
    nc = bacc.Bacc("TRN2", target_bir_lowering=False, debug=False)

    d_x = nc.dram_tensor("x", (L, D), F32, kind="ExternalInput")
    d_pw = nc.dram_tensor("pw", (H, D), F32, kind="ExternalInput")
    d_lnw = nc.dram_tensor("lnw", (D,), F32, kind="ExternalInput")
    d_lnb = nc.dram_tensor("lnb", (D,), F32, kind="ExternalInput")
    d_cb = nc.dram_tensor("cb", (H, C), F32, kind="ExternalInput")
    d_cbt = nc.dram_tensor("cbt", (C, H), F32, kind="ExternalInput")
    d_lab = nc.dram_tensor("labels", (128, TT), I32, kind="ExternalOutput")

    with tile.TileContext(nc) as tc:
        with tc.tile_pool(name="consts", bufs=1) as consts, \
             tc.tile_pool(name="persist", bufs=1) as persist, \
             tc.tile_pool(name="dram", bufs=1, space="DRAM") as dram:
            scratch = dram.tile([C], F32)

            # ---------- constants ----------
            ident = consts.tile([128, 128], F32)
            make_identity(nc, ident)
            eps_t = consts.tile([128, 1], F32)
            nc.vector.memset(eps_t, LN_EPS)
            lnw_bc = consts.tile([128, D], F32)
            nc.sync.dma_start(
                out=lnw_bc,
                in_=bass.AP(tensor=d_lnw, offset=0, ap=[[0, 128], [1, D]]))
            lnb_bc = consts.tile([128, D], F32)
            nc.sync.dma_start(
                out=lnb_bc,
                in_=bass.AP(tensor=d_lnb, offset=0, ap=[[0, 128], [1, D]]))
            chunk_off = consts.tile([128, CCH], F32)
            for j in range(CCH):
                nc.vector.memset(chunk_off[:, j:j + 1], 512.0 * j)

            # persistent fp16 split of t^T: (h, tok) layout
            th = [persist.tile([128, L], F16, name=f"th{h}", tag=f"th{h}")
                  for h in range(HT)]
            tl = [persist.tile([128, L], F16, name=f"tl{h}", tag=f"tl{h}")
                  for h in range(HT)]

            # ---------- phase A: LN + transposes + projection + split ----------
            with tc.tile_pool(name="phA", bufs=1) as phA, \
                 tc.tile_pool(name="ldtmp", bufs=3) as ldtmp, \
                 tc.tile_pool(name="psA", bufs=2, space="PSUM") as psA, \
                 tc.tile_pool(name="psTr", bufs=2, space="PSUM") as psTr:

                # proj weight: load (h,d), PE-transpose to (d,h)
                pwT = [phA.tile([128, H], F32, name=f"pwT{d}", tag=f"pwT{d}")
                       for d in range(DT)]
                for h in range(HT):
                    pw_t = ldtmp.tile([128, D], F32, tag="pw_t")
                    nc.sync.dma_start(out=pw_t, in_=d_pw[h * 128:(h + 1) * 128, :])
                    for d in range(DT):
                        ps_tr = psTr.tile([128, 128], F32, tag="ps_tr")
                        nc.tensor.transpose(ps_tr, pw_t[:, d * 128:(d + 1) * 128],
                                            ident)
                        nc.scalar.copy(out=pwT[d][:, h * 128:(h + 1) * 128],
                                       in_=ps_tr)

                # LayerNorm + transpose to xnT (d, tok)
                xnT = [phA.tile([128, L], F32, name=f"xnT{d}", tag=f"xnT{d}")
                       for d in range(DT)]
                for t in range(TT):
                    x_t = ldtmp.tile([128, D], F32, tag="x_t")
                    nc.sync.dma_start(out=x_t, in_=d_x[t * 128:(t + 1) * 128, :])
                    stats = ldtmp.tile([128, 6], F32, tag="stats")
                    nc.vector.bn_stats(out=stats, in_=x_t)
                    mv = ldtmp.tile([128, 2], F32, tag="mv")
                    nc.vector.bn_aggr(out=mv, in_=stats)
                    rstd = ldtmp.tile([128, 1], F32, tag="rstd")
                    nc.scalar.activation(out=rstd, in_=mv[:, 1:2],
                                         func=mybir.ActivationFunctionType.Sqrt,
                                         bias=eps_t, scale=1.0)
                    nc.vector.reciprocal(out=rstd, in_=rstd)
                    xn = ldtmp.tile([128, D], F32, tag="xn")
                    nc.vector.tensor_scalar(
                        out=xn, in0=x_t, scalar1=mv[:, 0:1], scalar2=rstd,
                        op0=mybir.AluOpType.subtract, op1=mybir.AluOpType.mult)
                    nc.vector.tensor_mul(out=xn, in0=xn, in1=lnw_bc)
                    nc.vector.tensor_add(out=xn, in0=xn, in1=lnb_bc)
                    for d in range(DT):
                        ps_tr = psTr.tile([128, 128], F32, tag="ps_tr")
                        nc.tensor.transpose(ps_tr, xn[:, d * 128:(d + 1) * 128],
                                            ident)
                        nc.scalar.copy(out=xnT[d][:, t * 128:(t + 1) * 128],
                                       in_=ps_tr)

                # projection t^T[h_tile, tok] = sum_d pwT[d,h].T @ xnT[d, tok]
                # fp32 (exact); split each PSUM into fp16 hi / lo*2^11.
                # tok-chunk outer so the cross phase can start on the first
                # token tiles while later chunks are still projecting.
                for tk in range(TOKC):
                    for h in range(HT):
                        ps_t = psA.tile([128, 512], F32, tag="ps_t", bufs=4)
                        for d in range(DT):
                            nc.tensor.matmul(
                                ps_t,
                                lhsT=pwT[d][:, h * 128:(h + 1) * 128],
                                rhs=xnT[d][:, tk * 512:(tk + 1) * 512],
                                start=(d == 0), stop=(d == DT - 1))
                        tsl = slice(tk * 512, (tk + 1) * 512)
                        nc.scalar.copy(out=th[h][:, tsl], in_=ps_t)
                        tmp = ldtmp.tile([128, 512], F32, tag="split_tmp")
                        nc.vector.tensor_sub(out=tmp, in0=ps_t, in1=th[h][:, tsl])
                        nc.scalar.activation(out=tl[h][:, tsl], in_=tmp,
                                             func=mybir.ActivationFunctionType.Copy,
                                             scale=SC)

            # ---------- phase B: cross matmul + per-chunk argmax ----------
            cval = [persist.tile([128, CCH], F32, name=f"cval{t}", tag=f"cval{t}")
                    for t in range(TT)]
            cidx = [persist.tile([128, CCH], U32, name=f"cidx{t}", tag=f"cidx{t}")
                    for t in range(TT)]

            with tc.tile_pool(name="cbf", bufs=1) as cbf_pool, \
                 tc.tile_pool(name="csplit", bufs=2) as csplit, \
                 tc.tile_pool(name="strips", bufs=4) as strips, \
                 tc.tile_pool(name="psB", bufs=5, space="PSUM") as psB:

                for cc in range(CCH):
                    csl = slice(cc * 512, (cc + 1) * 512)
                    cb_f = []
                    for h in range(HT):
                        t_ = cbf_pool.tile([128, 512], F32, name=f"cbf{h}",
                                           tag=f"cbf{h}")
                        nc.sync.dma_start(out=t_, in_=d_cb[h * 128:(h + 1) * 128,
                                                           csl])
                        cb_f.append(t_)
                    # bias_cc = 1024 * sum_h c^2 for this chunk's codewords:
                    # square + free-dim reduce over cbT rows (c on partitions),
                    # then a DRAM bounce to re-layout as (128 bcast, 512 c).
                    csq_cols = csplit.tile([128, 4], F32, name="csq_cols",
                                           tag="csq_cols")
                    for j in range(4):
                        cbt_t = csplit.tile([128, H], F32, name="cbt_t",
                                            tag="cbt_t", bufs=3)
                        nc.sync.dma_start(
                            out=cbt_t,
                            in_=d_cbt[cc * 512 + j * 128:cc * 512 + (j + 1) * 128, :])
                        sq_t = csplit.tile([128, H], F32, name="sq_t",
                                           tag="sq_t", bufs=3)
                        nc.scalar.activation(out=sq_t, in_=cbt_t,
                                             func=mybir.ActivationFunctionType.Square)
                        nc.vector.tensor_reduce(
                            out=csq_cols[:, j:j + 1], in_=sq_t,
                            axis=mybir.AxisListType.X, op=mybir.AluOpType.add)
                    nc.vector.tensor_scalar_mul(csq_cols, csq_cols, SC * 0.5)
                    nc.sync.dma_start(
                        out=bass.AP(tensor=scratch.tensor, offset=scratch.offset
                                    + cc * 512, ap=[[1, 128], [128, 4]]),
                        in_=csq_cols)
                    bias_cc = csplit.tile([128, 512], F32, name="bias_cc",
                                          tag="bias_cc")
                    nc.sync.dma_start(
                        out=bias_cc,
                        in_=bass.AP(tensor=scratch.tensor, offset=scratch.offset
                                    + cc * 512, ap=[[0, 128], [1, 512]]))
                    ch, chs, cls = [], [], []
                    for h in range(HT):
                        ch_t = csplit.tile([128, 512], F16, name=f"ch{h}",
                                           tag=f"ch{h}")
                        nc.scalar.copy(out=ch_t, in_=cb_f[h])
                        chs_t = csplit.tile([128, 512], F16, name=f"chs{h}",
                                            tag=f"chs{h}")
                        nc.scalar.activation(out=chs_t, in_=cb_f[h],
                                             func=mybir.ActivationFunctionType.Copy,
                                             scale=SC)
                        tmpc = strips.tile([128, 512], F32, tag="tmpc")
                        nc.vector.tensor_sub(out=tmpc, in0=cb_f[h], in1=ch_t)
                        cls_t = csplit.tile([128, 512], F16, name=f"cls{h}",
                                            tag=f"cls{h}")
                        nc.scalar.activation(out=cls_t, in_=tmpc,
                                             func=mybir.ActivationFunctionType.Copy,
                                             scale=SC)
                        ch.append(ch_t)
                        chs.append(chs_t)
                        cls.append(cls_t)

                    for t in range(TT):
                        tsl = slice(t * 128, (t + 1) * 128)
                        acc = psB.tile([128, 512], F32, tag="acc")
                        if passes == 3:
                            for h in range(HT):
                                nc.tensor.matmul(acc, lhsT=th[h][:, tsl],
                                                 rhs=chs[h], start=(h == 0),
                                                 stop=False)
                                nc.tensor.matmul(acc, lhsT=th[h][:, tsl],
                                                 rhs=cls[h], start=False,
                                                 stop=False)
                            for h in range(HT):
                                nc.tensor.matmul(acc, lhsT=tl[h][:, tsl],
                                                 rhs=ch[h], start=False,
                                                 stop=(h == HT - 1))
                        elif passes == 2:
                            for h in range(HT):
                                nc.tensor.matmul(acc, lhsT=th[h][:, tsl],
                                                 rhs=chs[h], start=(h == 0),
                                                 stop=False)
                            for h in range(HT):
                                nc.tensor.matmul(acc, lhsT=tl[h][:, tsl],
                                                 rhs=ch[h], start=False,
                                                 stop=(h == HT - 1))
                        else:
                            for h in range(HT):
                                nc.tensor.matmul(acc, lhsT=th[h][:, tsl],
                                                 rhs=chs[h], start=(h == 0),
                                                 stop=(h == HT - 1))
                        s = strips.tile([128, 512], F32, tag="s")
                        nc.vector.tensor_sub(out=s, in0=acc, in1=bias_cc)
                        mx8 = strips.tile([128, 8], F32, tag="mx8", bufs=6)
                        nc.vector.max(out=mx8, in_=s)
                        ix8 = strips.tile([128, 8], U32, tag="ix8", bufs=6)
                        nc.vector.max_index(out=ix8, in_max=mx8, in_values=s)
                        nc.gpsimd.tensor_copy(out=cval[t][:, cc:cc + 1],
                                              in_=mx8[:, 0:1])
                        nc.gpsimd.tensor_copy(out=cidx[t][:, cc:cc + 1],
                                              in_=ix8[:, 0:1])

            # ---------- phase C: combine the 16 chunk winners ----------
            with tc.tile_pool(name="fin", bufs=2) as fin:
                for t in range(TT):
                    cidxf = fin.tile([128, CCH], F32, tag="cidxf")
                    nc.vector.tensor_copy(cidxf, cidx[t])
                    gmx = fin.tile([128, 8], F32, tag="gmx")
                    nc.vector.max(out=gmx, in_=cval[t])
                    mask = fin.tile([128, CCH], F32, tag="mask")
                    nc.vector.tensor_scalar(
                        out=mask, in0=cval[t], scalar1=gmx[:, 0:1], scalar2=None,
                        op0=mybir.AluOpType.is_ge)
                    inv = fin.tile([128, CCH], F32, tag="inv")
                    nc.vector.tensor_scalar(
                        out=inv, in0=mask, scalar1=-16384.0, scalar2=16384.0,
                        op0=mybir.AluOpType.mult, op1=mybir.AluOpType.add)
                    cand = fin.tile([128, CCH], F32, tag="cand")
                    nc.vector.tensor_add(cand, cidxf, chunk_off)
                    nc.vector.tensor_add(cand, cand, inv)
                    win = fin.tile([128, 1], F32, tag="win")
                    nc.vector.tensor_reduce(out=win, in_=cand,
                                            axis=mybir.AxisListType.X,
                                            op=mybir.AluOpType.min)
                    lab = fin.tile([128, 1], I32, tag="lab")
                    nc.vector.tensor_copy(lab, win)
                    nc.sync.dma_start(out=d_lab[:, t:t + 1], in_=lab)

    nc.compile()
    return nc


_NC_CACHE = None


def make_in_maps(inputs):
    input_values = np.ascontiguousarray(inputs["input_values"], np.float32)
    pw = np.ascontiguousarray(inputs["proj_weight"], np.float32)
    lnw = np.ascontiguousarray(inputs["ln_weight"], np.float32)
    lnb = np.ascontiguousarray(inputs["ln_bias"], np.float32)
    cb = np.ascontiguousarray(inputs["codebook"], np.float32)
    cbt = np.ascontiguousarray(cb.T)

    in_maps = []
    for i in range(N_CORES):
        in_maps.append({
            "x": np.ascontiguousarray(input_values[i]),
            "pw": pw, "lnw": lnw, "lnb": lnb, "cb": cb, "cbt": cbt,
        })
    return in_maps


def kernel(input_values, ln_weight, ln_bias, proj_weight, codebook):
    global _NC_CACHE
    if _NC_CACHE is None:
        _NC_CACHE = build_nc()
    nc = _NC_CACHE

    in_maps = make_in_maps(dict(
        input_values=input_values, ln_weight=ln_weight, ln_bias=ln_bias,
        proj_weight=proj_weight, codebook=codebook))
    res = run_bass_kernel_spmd(nc, in_maps, core_ids=list(range(N_CORES)))
    out = np.empty((B, L), np.int32)
    for i in range(N_CORES):
        out[i] = res.results[i]["labels"].T.reshape(L)
    return out



# revision 3
# speedup vs baseline: 2.3138x; 2.3138x over previous
"""Trainium2 Bass kernel for BestRQ vector-quantization codebook lookup.

Pipeline (per NeuronCore, data-parallel over batch):
  x (2048,512) --LayerNorm--> xn --PE transpose--> xnT (d-major)
  t^T = projW^T @ xn^T  (fp32 matmul, accumulated over d)
  t split into fp16 hi (th16) and fp8-e4m3 digits for corrections.
  codebook streamed in 512-column chunks as fp16 hi (chs = c*2048) and
  fp8 digits.
  score*2048 accumulates in one PSUM group per (chunk, token-tile):
    main:      th16 @ chs16                  (8 fp16 matmuls, h-blocks)
    t-corr:    tl8 @ ch8                     (4 fp8 DoubleRow matmuls)
    c-corr:    th8 @ cls8  [scheme fp8corr2] (4 fp8 DoubleRow matmuls)
  s = score - 1024*||c||^2 ; per-chunk argmax via DVE max8/max_index;
  global combine over 16 chunks.

Schemes (host-emulated label flips / rel-err vs the fp32 reference):
  fp16x3   3 fp16 passes (original baseline)   1 flip   8.8e-4
  fp8corr2 fp16 main + both fp8 DR corrections 2 flips  9.7e-4
  fp8corr  fp16 main + t-side fp8 DR corr      5 flips  8.4e-3
  fp16x2   2 fp16 passes                       5 flips  8.4e-3
"""

import numpy as np

import concourse.bacc as bacc
import concourse.bass as bass
import concourse.mybir as mybir
import concourse.tile as tile
from concourse.bass_utils import run_bass_kernel_spmd
from concourse.masks import make_identity

B, L, D, H, C = 8, 2048, 512, 1024, 8192
LN_EPS = 1e-5
N_CORES = 8

TT = L // 128      # 16 token tiles
CCH = C // 512     # 16 codebook chunks
HT = H // 128      # 8 h tiles
DT = D // 128      # 4 d tiles
TOKC = L // 512    # 4 token chunks (projection)
SC = 2048.0        # 2^11 lo-part scale

F32 = mybir.dt.float32
F16 = mybir.dt.float16
F8 = mybir.dt.float8e4
I32 = mybir.dt.int32
U32 = mybir.dt.uint32

SCHEME = "fp8corr2"


def _emit(nc, tc, d, scheme, rep):
    """Emit one full pipeline iteration into the TileContext."""
    DR = mybir.MatmulPerfMode.DoubleRow
    d_x, d_pw, d_lnw, d_lnb, d_cb, d_cbt, d_lab = (
        d["x"], d["pw"], d["lnw"], d["lnb"], d["cb"], d["cbt"], d["lab"])
    use_fp8 = scheme in ("fp8corr", "fp8corr2")
    csplit_blocks = 16 if scheme == "fp8corr2" else 8
    passes = {"fp16x3": 3, "fp16x2": 2}.get(scheme, 0)

    with tc.tile_pool(name=f"consts{rep}", bufs=1) as consts, \
         tc.tile_pool(name=f"persist{rep}", bufs=1) as persist, \
         tc.tile_pool(name=f"dram{rep}", bufs=1, space="DRAM") as dram:
        scratch = dram.tile([C], F32)

        # ---------- constants ----------
        ident = consts.tile([128, 128], F32)
        make_identity(nc, ident)
        eps_t = consts.tile([128, 1], F32)
        nc.vector.memset(eps_t, LN_EPS)
        lnw_bc = consts.tile([128, D], F32)
        nc.sync.dma_start(
            out=lnw_bc,
            in_=bass.AP(tensor=d_lnw, offset=0, ap=[[0, 128], [1, D]]))
        lnb_bc = consts.tile([128, D], F32)
        nc.sync.dma_start(
            out=lnb_bc,
            in_=bass.AP(tensor=d_lnb, offset=0, ap=[[0, 128], [1, D]]))
        chunk_off = consts.tile([128, CCH], F32)
        for j in range(CCH):
            nc.vector.memset(chunk_off[:, j:j + 1], 512.0 * j)

        # persistent t^T tiles: fp16 hi + correction digits
        th = [persist.tile([128, L], F16, name=f"th{h}_{rep}", tag=f"th{h}")
              for h in range(HT)]
        if use_fp8:
            t8 = persist.tile([128, csplit_blocks, L], F8, name=f"t8_{rep}",
                              tag="t8")
            tl = None
        else:
            t8 = None
            tl = [persist.tile([128, L], F16, name=f"tl{h}_{rep}",
                               tag=f"tl{h}") for h in range(HT)]

        # ---------- phase A: LN + transposes + projection + split ----------
        with tc.tile_pool(name=f"phA{rep}", bufs=1) as phA, \
             tc.tile_pool(name=f"ldtmp{rep}", bufs=3) as ldtmp, \
             tc.tile_pool(name=f"psA{rep}", bufs=2, space="PSUM") as psA, \
             tc.tile_pool(name=f"psTr{rep}", bufs=2, space="PSUM") as psTr:

            # proj weight: load (h,d), PE-transpose to (d,h)
            pwT = [phA.tile([128, H], F32, name=f"pwT{dd}_{rep}",
                            tag=f"pwT{dd}") for dd in range(DT)]
            for h in range(HT):
                pw_t = ldtmp.tile([128, D], F32, tag="pw_t")
                nc.sync.dma_start(out=pw_t, in_=d_pw[h * 128:(h + 1) * 128, :])
                for dd in range(DT):
                    ps_tr = psTr.tile([128, 128], F32, tag="ps_tr")
                    nc.tensor.transpose(ps_tr, pw_t[:, dd * 128:(dd + 1) * 128],
                                        ident)
                    nc.scalar.copy(out=pwT[dd][:, h * 128:(h + 1) * 128],
                                   in_=ps_tr)

            # LayerNorm + transpose to xnT (d, tok)
            xnT = [phA.tile([128, L], F32, name=f"xnT{dd}_{rep}",
                            tag=f"xnT{dd}") for dd in range(DT)]
            for t in range(TT):
                x_t = ldtmp.tile([128, D], F32, tag="x_t")
                nc.sync.dma_start(out=x_t, in_=d_x[t * 128:(t + 1) * 128, :])
                stats = ldtmp.tile([128, 6], F32, tag="stats")
                nc.vector.bn_stats(out=stats, in_=x_t)
                mv = ldtmp.tile([128, 2], F32, tag="mv")
                nc.vector.bn_aggr(out=mv, in_=stats)
                rstd = ldtmp.tile([128, 1], F32, tag="rstd")
                nc.scalar.activation(out=rstd, in_=mv[:, 1:2],
                                     func=mybir.ActivationFunctionType.Sqrt,
                                     bias=eps_t, scale=1.0)
                nc.vector.reciprocal(out=rstd, in_=rstd)
                xn = ldtmp.tile([128, D], F32, tag="xn")
                nc.vector.tensor_scalar(
                    out=xn, in0=x_t, scalar1=mv[:, 0:1], scalar2=rstd,
                    op0=mybir.AluOpType.subtract, op1=mybir.AluOpType.mult)
                nc.vector.tensor_mul(out=xn, in0=xn, in1=lnw_bc)
                nc.vector.tensor_add(out=xn, in0=xn, in1=lnb_bc)
                for dd in range(DT):
                    ps_tr = psTr.tile([128, 128], F32, tag="ps_tr")
                    nc.tensor.transpose(ps_tr, xn[:, dd * 128:(dd + 1) * 128],
                                        ident)
                    nc.scalar.copy(out=xnT[dd][:, t * 128:(t + 1) * 128],
                                   in_=ps_tr)

            # projection t^T[h_tile, tok] = sum_d pwT[d,h].T @ xnT[d, tok]
            # fp32 (exact); split each PSUM strip into the scheme's digits.
            for tk in range(TOKC):
                for h in range(HT):
                    ps_t = psA.tile([128, 512], F32, tag="ps_t", bufs=4)
                    for dd in range(DT):
                        nc.tensor.matmul(
                            ps_t,
                            lhsT=pwT[dd][:, h * 128:(h + 1) * 128],
                            rhs=xnT[dd][:, tk * 512:(tk + 1) * 512],
                            start=(dd == 0), stop=(dd == DT - 1))
                    tsl = slice(tk * 512, (tk + 1) * 512)
                    nc.scalar.copy(out=th[h][:, tsl], in_=ps_t)
                    tmp = ldtmp.tile([128, 512], F32, tag="split_tmp")
                    nc.vector.tensor_sub(out=tmp, in0=ps_t, in1=th[h][:, tsl])
                    if use_fp8:
                        # tl8 digit: (t - th16) * SC, cast to e4m3
                        nc.scalar.activation(
                            out=t8[:, h, tsl], in_=tmp,
                            func=mybir.ActivationFunctionType.Copy, scale=SC)
                        if scheme == "fp8corr2":
                            nc.scalar.copy(out=t8[:, HT + h, tsl],
                                           in_=th[h][:, tsl])
                    else:
                        nc.scalar.activation(
                            out=tl[h][:, tsl], in_=tmp,
                            func=mybir.ActivationFunctionType.Copy, scale=SC)

        # ---------- phase B: cross matmul + per-chunk argmax ----------
        cval = [persist.tile([128, CCH], F32, name=f"cval{t}_{rep}",
                             tag=f"cval{t}") for t in range(TT)]
        cidx = [persist.tile([128, CCH], U32, name=f"cidx{t}_{rep}",
                             tag=f"cidx{t}") for t in range(TT)]

        with tc.tile_pool(name=f"cbf{rep}", bufs=1) as cbf_pool, \
             tc.tile_pool(name=f"csplit{rep}", bufs=2) as csplit, \
             tc.tile_pool(name=f"strips{rep}", bufs=4) as strips, \
             tc.tile_pool(name=f"psB{rep}", bufs=5, space="PSUM") as psB:

            for cc in range(CCH):
                csl = slice(cc * 512, (cc + 1) * 512)
                cb_f = []
                for h in range(HT):
                    t_ = cbf_pool.tile([128, 512], F32, name=f"cbf{h}",
                                       tag=f"cbf{h}")
                    nc.sync.dma_start(out=t_, in_=d_cb[h * 128:(h + 1) * 128,
                                                       csl])
                    cb_f.append(t_)
                # bias_cc = 1024 * sum_h c^2 for this chunk's codewords:
                # square + free-dim reduce over cbT rows (c on partitions),
                # then a DRAM bounce to re-layout as (128 bcast, 512 c).
                csq_cols = csplit.tile([128, 4], F32, name="csq_cols",
                                       tag="csq_cols")
                for j in range(4):
                    cbt_t = csplit.tile([128, H], F32, name="cbt_t",
                                        tag="cbt_t", bufs=3)
                    nc.sync.dma_start(
                        out=cbt_t,
                        in_=d_cbt[cc * 512 + j * 128:cc * 512 + (j + 1) * 128, :])
                    sq_t = csplit.tile([128, H], F32, name="sq_t",
                                       tag="sq_t", bufs=3)
                    nc.scalar.activation(out=sq_t, in_=cbt_t,
                                         func=mybir.ActivationFunctionType.Square)
                    nc.vector.tensor_reduce(
                        out=csq_cols[:, j:j + 1], in_=sq_t,
                        axis=mybir.AxisListType.X, op=mybir.AluOpType.add)
                nc.vector.tensor_scalar_mul(csq_cols, csq_cols, SC * 0.5)
                nc.sync.dma_start(
                    out=bass.AP(tensor=scratch.tensor, offset=scratch.offset
                                + cc * 512, ap=[[1, 128], [128, 4]]),
                    in_=csq_cols)
                bias_cc = csplit.tile([128, 512], F32, name="bias_cc",
                                      tag="bias_cc")
                nc.sync.dma_start(
                    out=bias_cc,
                    in_=bass.AP(tensor=scratch.tensor, offset=scratch.offset
                                + cc * 512, ap=[[0, 128], [1, 512]]))

                # fp16 hi rhs (chs = c*SC), plus scheme-dependent digits
                chs = []
                for h in range(HT):
                    chs_t = csplit.tile([128, 512], F16, name=f"chs{h}",
                                        tag=f"chs{h}")
                    nc.scalar.activation(out=chs_t, in_=cb_f[h],
                                         func=mybir.ActivationFunctionType.Copy,
                                         scale=SC)
                    chs.append(chs_t)
                if use_fp8:
                    cb8 = csplit.tile([128, csplit_blocks, 512], F8,
                                      name="cb8", tag="cb8")
                    for h in range(HT):
                        nc.scalar.copy(out=cb8[:, h, :], in_=cb_f[h])
                    if scheme == "fp8corr2":
                        for h in range(HT):
                            ch16 = strips.tile([128, 512], F16, tag="ch16")
                            nc.scalar.copy(out=ch16, in_=cb_f[h])
                            tmpc = strips.tile([128, 512], F32, tag="tmpc")
                            nc.vector.tensor_sub(out=tmpc, in0=cb_f[h],
                                                 in1=ch16)
                            nc.scalar.activation(
                                out=cb8[:, HT + h, :], in_=tmpc,
                                func=mybir.ActivationFunctionType.Copy,
                                scale=SC)
                    ch = cls = None
                else:
                    cb8 = None
                    ch, cls = [], []
                    for h in range(HT):
                        ch_t = csplit.tile([128, 512], F16, name=f"ch{h}",
                                           tag=f"ch{h}")
                        nc.scalar.copy(out=ch_t, in_=cb_f[h])
                        ch.append(ch_t)
                        if passes == 3:
                            tmpc = strips.tile([128, 512], F32, tag="tmpc")
                            nc.vector.tensor_sub(out=tmpc, in0=cb_f[h],
                                                 in1=ch_t)
                            cls_t = csplit.tile([128, 512], F16,
                                                name=f"cls{h}", tag=f"cls{h}")
                            nc.scalar.activation(
                                out=cls_t, in_=tmpc,
                                func=mybir.ActivationFunctionType.Copy,
                                scale=SC)
                            cls.append(cls_t)

                for t in range(TT):
                    tsl = slice(t * 128, (t + 1) * 128)
                    acc = psB.tile([128, 512], F32, tag="acc")
                    if use_fp8:
                        ndr = csplit_blocks // 2
                        for h in range(HT):
                            nc.tensor.matmul(acc, lhsT=th[h][:, tsl],
                                             rhs=chs[h], start=(h == 0),
                                             stop=False)
                        for p in range(ndr):
                            nc.tensor.matmul(
                                acc, lhsT=t8[:, 2 * p:2 * p + 2, tsl],
                                rhs=cb8[:, 2 * p:2 * p + 2, :],
                                start=False, stop=(p == ndr - 1),
                                perf_mode=DR)
                    elif passes == 3:
                        for h in range(HT):
                            nc.tensor.matmul(acc, lhsT=th[h][:, tsl],
                                             rhs=chs[h], start=(h == 0),
                                             stop=False)
                            nc.tensor.matmul(acc, lhsT=th[h][:, tsl],
                                             rhs=cls[h], start=False,
                                             stop=False)
                        for h in range(HT):
                            nc.tensor.matmul(acc, lhsT=tl[h][:, tsl],
                                             rhs=ch[h], start=False,
                                             stop=(h == HT - 1))
                    else:
                        for h in range(HT):
                            nc.tensor.matmul(acc, lhsT=th[h][:, tsl],
                                             rhs=chs[h], start=(h == 0),
                                             stop=False)
                        for h in range(HT):
                            nc.tensor.matmul(acc, lhsT=tl[h][:, tsl],
                                             rhs=ch[h], start=False,
                                             stop=(h == HT - 1))
                    s = strips.tile([128, 512], F32, tag="s")
                    nc.vector.tensor_sub(out=s, in0=acc, in1=bias_cc)
                    mx8 = strips.tile([128, 8], F32, tag="mx8", bufs=6)
                    nc.vector.max(out=mx8, in_=s)
                    ix8 = strips.tile([128, 8], U32, tag="ix8", bufs=6)
                    nc.vector.max_index(out=ix8, in_max=mx8, in_values=s)
                    nc.gpsimd.tensor_copy(out=cval[t][:, cc:cc + 1],
                                          in_=mx8[:, 0:1])
                    nc.gpsimd.tensor_copy(out=cidx[t][:, cc:cc + 1],
                                          in_=ix8[:, 0:1])

        # ---------- phase C: combine the 16 chunk winners ----------
        with tc.tile_pool(name=f"fin{rep}", bufs=2) as fin:
            for t in range(TT):
                cidxf = fin.tile([128, CCH], F32, tag="cidxf")
                nc.vector.tensor_copy(cidxf, cidx[t])
                gmx = fin.tile([128, 8], F32, tag="gmx")
                nc.vector.max(out=gmx, in_=cval[t])
                mask = fin.tile([128, CCH], F32, tag="mask")
                nc.vector.tensor_scalar(
                    out=mask, in0=cval[t], scalar1=gmx[:, 0:1], scalar2=None,
                    op0=mybir.AluOpType.is_ge)
                inv = fin.tile([128, CCH], F32, tag="inv")
                nc.vector.tensor_scalar(
                    out=inv, in0=mask, scalar1=-16384.0, scalar2=16384.0,
                    op0=mybir.AluOpType.mult, op1=mybir.AluOpType.add)
                cand = fin.tile([128, CCH], F32, tag="cand")
                nc.vector.tensor_add(cand, cidxf, chunk_off)
                nc.vector.tensor_add(cand, cand, inv)
                win = fin.tile([128, 1], F32, tag="win")
                nc.vector.tensor_reduce(out=win, in_=cand,
                                        axis=mybir.AxisListType.X,
                                        op=mybir.AluOpType.min)
                lab = fin.tile([128, 1], I32, tag="lab")
                nc.vector.tensor_copy(lab, win)
                nc.sync.dma_start(out=d_lab[:, t:t + 1], in_=lab)


def build_nc(scheme=SCHEME, repeat=1):
    nc = bacc.Bacc("TRN2", target_bir_lowering=False, debug=False)

    d = dict(
        x=nc.dram_tensor("x", (L, D), F32, kind="ExternalInput"),
        pw=nc.dram_tensor("pw", (H, D), F32, kind="ExternalInput"),
        lnw=nc.dram_tensor("lnw", (D,), F32, kind="ExternalInput"),
        lnb=nc.dram_tensor("lnb", (D,), F32, kind="ExternalInput"),
        cb=nc.dram_tensor("cb", (H, C), F32, kind="ExternalInput"),
        cbt=nc.dram_tensor("cbt", (C, H), F32, kind="ExternalInput"),
        lab=nc.dram_tensor("labels", (128, TT), I32, kind="ExternalOutput"),
    )

    with tile.TileContext(nc) as tc:
        for rep in range(repeat):
            _emit(nc, tc, d, scheme, rep)

    nc.compile()
    return nc


_NC_CACHE = {}


def make_in_maps(inputs):
    input_values = np.ascontiguousarray(inputs["input_values"], np.float32)
    pw = np.ascontiguousarray(inputs["proj_weight"], np.float32)
    lnw = np.ascontiguousarray(inputs["ln_weight"], np.float32)
    lnb = np.ascontiguousarray(inputs["ln_bias"], np.float32)
    cb = np.ascontiguousarray(inputs["codebook"], np.float32)
    cbt = np.ascontiguousarray(cb.T)

    in_maps = []
    for i in range(N_CORES):
        in_maps.append({
            "x": np.ascontiguousarray(input_values[i]),
            "pw": pw, "lnw": lnw, "lnb": lnb, "cb": cb, "cbt": cbt,
        })
    return in_maps


def kernel(input_values, ln_weight, ln_bias, proj_weight, codebook):
    key = (SCHEME, 1)
    if key not in _NC_CACHE:
        _NC_CACHE[key] = build_nc(SCHEME, 1)
    nc = _NC_CACHE[key]

    in_maps = make_in_maps(dict(
        input_values=input_values, ln_weight=ln_weight, ln_bias=ln_bias,
        proj_weight=proj_weight, codebook=codebook))
    res = run_bass_kernel_spmd(nc, in_maps, core_ids=list(range(N_CORES)))
    out = np.empty((B, L), np.int32)
    for i in range(N_CORES):
        out[i] = res.results[i]["labels"].T.reshape(L)
    return out


# revision 22
# speedup vs baseline: 2.9409x; 1.2710x over previous
"""Trainium2 Bass kernel for BestRQ vector-quantization codebook lookup.

Pipeline (per NeuronCore, data-parallel over batch):
  x (2048,512) --LayerNorm--> xn --PE transpose--> xnT (d-major)
  t^T = projW^T @ xn^T  (fp32 matmul, accumulated over d)
  t split into fp16 hi (th16) and fp8-e4m3 digits for corrections.
  codebook streamed in 512-column chunks as fp16 hi (chs = c*2048) and
  fp8 digits.
  score*2048 accumulates in one PSUM group per (chunk, token-tile):
    main:      th16 @ chs16                  (8 fp16 matmuls, h-blocks)
    t-corr:    tl8 @ ch8                     (4 fp8 DoubleRow matmuls)
    c-corr:    th8 @ cls8  [scheme fp8corr2] (4 fp8 DoubleRow matmuls)
  s = score - 1024*||c||^2 ; per-chunk argmax via DVE max8/max_index;
  vectorized global combine over the 16 chunks at the end.

Schemes (host-emulated label flips / rel-err vs the fp32 reference):
  fp16x3   3 fp16 passes (original baseline)   1 flip   8.8e-4
  fp8corr2 fp16 main + both fp8 DR corrections 2 flips  9.7e-4
  fp8corr  fp16 main + t-side fp8 DR corr      5 flips  8.4e-3
  fp16x2   2 fp16 passes                       5 flips  8.4e-3
"""

import numpy as np

import concourse.bacc as bacc
import concourse.bass as bass
import concourse.mybir as mybir
import concourse.tile as tile
from concourse.bass_utils import run_bass_kernel_spmd
from concourse.masks import make_identity

B, L, D, H, C = 8, 2048, 512, 1024, 8192
LN_EPS = 1e-5
N_CORES = 8

TT = L // 128      # 16 token tiles
CCH = C // 512     # 16 codebook chunks
HT = H // 128      # 8 h tiles
DT = D // 128      # 4 d tiles
TOKC = L // 512    # 4 token chunks (projection)
SC = 2048.0        # 2^11 lo-part scale

F32 = mybir.dt.float32
F16 = mybir.dt.float16
F8 = mybir.dt.float8e4
I32 = mybir.dt.int32
U32 = mybir.dt.uint32

SCHEME = "fp8corr"


def _emit(nc, tc, d, scheme, rep):
    """Emit one full pipeline iteration into the TileContext."""
    DR = mybir.MatmulPerfMode.DoubleRow
    d_x, d_pw, d_lnw, d_lnb, d_cb, d_cbt, d_lab = (
        d["x"], d["pw"], d["lnw"], d["lnb"], d["cb"], d["cbt"], d["lab"])
    ablate_A = scheme.endswith("_A")
    if ablate_A:
        scheme = scheme[:-2]
    use_fp8 = scheme in ("fp8corr", "fp8corr2")
    csplit_blocks = 16 if scheme == "fp8corr2" else 8
    passes = {"fp16x3": 3, "fp16x2": 2}.get(scheme, 0)

    with tc.tile_pool(name=f"consts{rep}", bufs=1) as consts, \
         tc.tile_pool(name=f"persist{rep}", bufs=1) as persist, \
         tc.tile_pool(name=f"dram{rep}", bufs=1, space="DRAM") as dram:
        scratch = dram.tile([C], F32)

        # ---------- constants ----------
        ident = consts.tile([128, 128], F32)
        make_identity(nc, ident)
        eps_t = consts.tile([128, 1], F32)
        nc.vector.memset(eps_t, LN_EPS)
        lnw_bc = consts.tile([128, D], F32)
        nc.sync.dma_start(
            out=lnw_bc,
            in_=bass.AP(tensor=d_lnw, offset=0, ap=[[0, 128], [1, D]]))
        lnb_bc = consts.tile([128, D], F32)
        nc.sync.dma_start(
            out=lnb_bc,
            in_=bass.AP(tensor=d_lnb, offset=0, ap=[[0, 128], [1, D]]))
        chunk_off = consts.tile([128, CCH], F32)
        for j in range(CCH):
            nc.vector.memset(chunk_off[:, j:j + 1], 512.0 * j)

        # persistent t^T tiles: fp16 hi + correction digits
        th = [persist.tile([128, L], F16, name=f"th{h}_{rep}", tag=f"th{h}")
              for h in range(HT)]
        if use_fp8:
            t8 = persist.tile([128, csplit_blocks, L], F8, name=f"t8_{rep}",
                              tag="t8")
            tl = None
        else:
            t8 = None
            tl = [persist.tile([128, L], F16, name=f"tl{h}_{rep}",
                               tag=f"tl{h}") for h in range(HT)]

        # ---------- phase A: LN + transposes + projection + split ----------
        # projection scheme (fp8 variants): fp16 main + fp8-DR corrections.
        #   t = xnh16 @ pwh16  +  2^-17 * (xnl8 @ pwh8[*64] + xnh8 @ pwl8)
        # where xnl8 = fp8((xn - xnh16)*SC), pwh8 = fp8(pwh*64),
        #       pwl8 = fp8((pw - pwh16)*SC*64), xnh8 = fp8(xn).
        # Both corrections arrive at scale 64*SC = 2^17 and accumulate in
        # one PSUM group; |t| error ~2^-15 (negligible vs the cross scheme).
        with tc.tile_pool(name=f"phA{rep}", bufs=1) as phA, \
             tc.tile_pool(name=f"ldtmp{rep}", bufs=3) as ldtmp:
            if use_fp8:
                pwh16 = [phA.tile([128, H], F16, name=f"pwh16_{dd}_{rep}",
                                  tag=f"pwh16_{dd}") for dd in range(DT)]
                pw8 = phA.tile([128, 2 * DT, H], F8, name=f"pw8_{rep}",
                               tag="pw8")
                xnh16 = [phA.tile([128, L], F16, name=f"xnh16_{dd}_{rep}",
                                  tag=f"xnh16_{dd}") for dd in range(DT)]
                xn8 = phA.tile([128, 2 * DT, L], F8, name=f"xn8_{rep}",
                               tag="xn8")
                pwT = xnT = None
            else:
                pwT = [phA.tile([128, H], F32, name=f"pwT{dd}_{rep}",
                                tag=f"pwT{dd}") for dd in range(DT)]
                xnT = [phA.tile([128, L], F32, name=f"xnT{dd}_{rep}",
                                tag=f"xnT{dd}") for dd in range(DT)]

            def emit_proj_tk(tk, pool):
                """Emit projection groups for one 512-token chunk (fp8 path)."""
                for dd in range(DT):
                    nc.scalar.copy(
                        out=xn8[:, DT + dd, tk * 512:(tk + 1) * 512],
                        in_=xnh16[dd][:, tk * 512:(tk + 1) * 512])
                for h in range(HT):
                    tsl = slice(tk * 512, (tk + 1) * 512)
                    hsl = slice(h * 128, (h + 1) * 128)
                    ps_t = pool.tile([128, 512], F32, tag="ps_t", bufs=3)
                    for dd in range(DT):
                        nc.tensor.matmul(
                            ps_t, lhsT=pwh16[dd][:, hsl],
                            rhs=xnh16[dd][:, tsl],
                            start=(dd == 0), stop=(dd == DT - 1))
                    ps_c = pool.tile([128, 512], F32, tag="ps_c", bufs=3)
                    for p in range(DT):
                        nc.tensor.matmul(
                            ps_c,
                            lhsT=pw8[:, 2 * p:2 * p + 2, hsl],
                            rhs=xn8[:, 2 * p:2 * p + 2, tsl],
                            start=(p == 0), stop=(p == DT - 1),
                            perf_mode=DR)
                    corr = ldtmp.tile([128, 512], F32, tag="corr")
                    nc.scalar.activation(
                        out=corr, in_=ps_c,
                        func=mybir.ActivationFunctionType.Copy,
                        scale=1.0 / (64.0 * SC))
                    t_f = ldtmp.tile([128, 512], F32, tag="t_f")
                    nc.vector.tensor_add(out=t_f, in0=ps_t, in1=corr)
                    nc.scalar.copy(out=th[h][:, tsl], in_=t_f)
                    tmp = ldtmp.tile([128, 512], F32, tag="split_tmp")
                    nc.vector.tensor_sub(out=tmp, in0=t_f, in1=th[h][:, tsl])
                    nc.scalar.activation(
                        out=t8[:, h, tsl], in_=tmp,
                        func=mybir.ActivationFunctionType.Copy, scale=SC)
                    if scheme == "fp8corr2":
                        nc.scalar.copy(out=t8[:, HT + h, tsl],
                                       in_=th[h][:, tsl])

            with tc.tile_pool(name=f"psTr{rep}", bufs=(2 if use_fp8 else 4),
                              space="PSUM") as psTr:
                # proj weight: load (h,d), PE-transpose to (d,h); split
                # straight off the transpose PSUM (fp8 path keeps no fp32
                # d-major copy; the fp8-hi casts happen in bulk later).
                for h in range(HT):
                    pw_t = ldtmp.tile([128, D], F32, tag="pw_t")
                    nc.sync.dma_start(out=pw_t,
                                      in_=d_pw[h * 128:(h + 1) * 128, :])
                    for dd in range(DT):
                        ps_tr = psTr.tile([128, 128], F32, tag="ps_tr")
                        nc.tensor.transpose(
                            ps_tr, pw_t[:, dd * 128:(dd + 1) * 128], ident)
                        hsl = slice(h * 128, (h + 1) * 128)
                        if use_fp8:
                            nc.scalar.copy(out=pwh16[dd][:, hsl], in_=ps_tr)
                            tmpw = ldtmp.tile([128, 128], F32, tag="tmpw")
                            nc.vector.tensor_sub(out=tmpw, in0=ps_tr,
                                                 in1=pwh16[dd][:, hsl])
                            nc.scalar.activation(
                                out=pw8[:, DT + dd, hsl], in_=tmpw,
                                func=mybir.ActivationFunctionType.Copy,
                                scale=SC * 64.0)
                        else:
                            nc.scalar.copy(out=pwT[dd][:, hsl], in_=ps_tr)
                if use_fp8:
                    for dd in range(DT):
                        nc.scalar.activation(
                            out=pw8[:, dd, :], in_=pwh16[dd],
                            func=mybir.ActivationFunctionType.Copy, scale=64.0)

                # LayerNorm + transpose to d-major; split off transpose PSUM
                for t in range(TT):
                    x_t = ldtmp.tile([128, D], F32, tag="x_t", bufs=6)
                    nc.sync.dma_start(out=x_t,
                                      in_=d_x[t * 128:(t + 1) * 128, :])
                    stats = ldtmp.tile([128, 6], F32, tag="stats")
                    nc.vector.bn_stats(out=stats, in_=x_t)
                    mv = ldtmp.tile([128, 2], F32, tag="mv")
                    nc.vector.bn_aggr(out=mv, in_=stats)
                    rstd = ldtmp.tile([128, 1], F32, tag="rstd")
                    nc.scalar.activation(out=rstd, in_=mv[:, 1:2],
                                         func=mybir.ActivationFunctionType.Sqrt,
                                         bias=eps_t, scale=1.0)
                    nc.vector.reciprocal(out=rstd, in_=rstd)
                    xn = ldtmp.tile([128, D], F32, tag="xn")
                    nc.vector.tensor_scalar(
                        out=xn, in0=x_t, scalar1=mv[:, 0:1], scalar2=rstd,
                        op0=mybir.AluOpType.subtract, op1=mybir.AluOpType.mult)
                    nc.vector.tensor_mul(out=xn, in0=xn, in1=lnw_bc)
                    nc.vector.tensor_add(out=xn, in0=xn, in1=lnb_bc)
                    for dd in range(DT):
                        ps_tr = psTr.tile([128, 128], F32, tag="ps_tr")
                        nc.tensor.transpose(
                            ps_tr, xn[:, dd * 128:(dd + 1) * 128], ident)
                        tsl128 = slice(t * 128, (t + 1) * 128)
                        if use_fp8:
                            nc.scalar.copy(out=xnh16[dd][:, tsl128],
                                           in_=ps_tr)
                            tmpx = ldtmp.tile([128, 128], F32, tag="tmpx")
                            nc.vector.tensor_sub(out=tmpx, in0=ps_tr,
                                                 in1=xnh16[dd][:, tsl128])
                            nc.scalar.activation(
                                out=xn8[:, dd, tsl128], in_=tmpx,
                                func=mybir.ActivationFunctionType.Copy,
                                scale=SC)
                        else:
                            nc.scalar.copy(out=xnT[dd][:, tsl128], in_=ps_tr)
                    if use_fp8 and t % 4 == 3:
                        emit_proj_tk(t // 4, psTr)

            # non-fp8 projection (fp8 path emits proj inline above)
            if not use_fp8:
                with tc.tile_pool(name=f"psA{rep}", bufs=2,
                                  space="PSUM") as psA:
                    for tk in range(TOKC):
                        for h in range(HT):
                            tsl = slice(tk * 512, (tk + 1) * 512)
                            hsl = slice(h * 128, (h + 1) * 128)
                            ps_t = psA.tile([128, 512], F32, tag="ps_t",
                                            bufs=4)
                            for dd in range(DT):
                                nc.tensor.matmul(
                                    ps_t,
                                    lhsT=pwT[dd][:, hsl],
                                    rhs=xnT[dd][:, tsl],
                                    start=(dd == 0), stop=(dd == DT - 1))
                            nc.scalar.copy(out=th[h][:, tsl], in_=ps_t)
                            tmp = ldtmp.tile([128, 512], F32, tag="split_tmp")
                            nc.vector.tensor_sub(out=tmp, in0=ps_t,
                                                 in1=th[h][:, tsl])
                            nc.scalar.activation(
                                out=tl[h][:, tsl], in_=tmp,
                                func=mybir.ActivationFunctionType.Copy,
                                scale=SC)

        # ---------- phase B: cross matmul + per-chunk argmax ----------
        if ablate_A:
            # phase-A timing ablation: emit a trivial output write and stop
            with tc.tile_pool(name=f"abl{rep}", bufs=1) as abl:
                z = abl.tile([128, TT], I32)
                nc.vector.tensor_copy(z, th[0][:, 0:TT])
                nc.sync.dma_start(out=d_lab[:, :], in_=z)
            return
        cval_all = persist.tile([128, TT, CCH], F32, name=f"cval_{rep}",
                                tag="cval")
        cidx_all = persist.tile([128, TT, CCH], F32, name=f"cidx_{rep}",
                                tag="cidx")

        with tc.tile_pool(name=f"cbf{rep}", bufs=1) as cbf_pool, \
             tc.tile_pool(name=f"csplit{rep}", bufs=3) as csplit, \
             tc.tile_pool(name=f"strips{rep}", bufs=4) as strips, \
             tc.tile_pool(name=f"psB{rep}", bufs=8, space="PSUM") as psB:

            for cc in range(CCH):
                csl = slice(cc * 512, (cc + 1) * 512)
                cb_f = []
                for h in range(HT):
                    t_ = cbf_pool.tile([128, 512], F32, name=f"cbf{h}",
                                       tag=f"cbf{h}")
                    nc.sync.dma_start(out=t_, in_=d_cb[h * 128:(h + 1) * 128,
                                                       csl])
                    cb_f.append(t_)
                # bias_cc = 1024 * sum_h c^2 for this chunk's codewords:
                # square + free-dim reduce over cbT rows (c on partitions),
                # then a DRAM bounce to re-layout as (128 bcast, 512 c).
                csq_cols = csplit.tile([128, 4], F32, name="csq_cols",
                                       tag="csq_cols")
                for j in range(4):
                    cbt_t = csplit.tile([128, H], F32, name="cbt_t",
                                        tag="cbt_t", bufs=3)
                    nc.sync.dma_start(
                        out=cbt_t,
                        in_=d_cbt[cc * 512 + j * 128:cc * 512 + (j + 1) * 128, :])
                    sq_t = csplit.tile([128, H], F32, name="sq_t",
                                       tag="sq_t", bufs=3)
                    nc.scalar.activation(out=sq_t, in_=cbt_t,
                                         func=mybir.ActivationFunctionType.Square)
                    nc.vector.tensor_reduce(
                        out=csq_cols[:, j:j + 1], in_=sq_t,
                        axis=mybir.AxisListType.X, op=mybir.AluOpType.add)
                nc.vector.tensor_scalar_mul(csq_cols, csq_cols, SC * 0.5)
                nc.sync.dma_start(
                    out=bass.AP(tensor=scratch.tensor, offset=scratch.offset
                                + cc * 512, ap=[[1, 128], [128, 4]]),
                    in_=csq_cols)
                bias_cc = csplit.tile([128, 512], F32, name="bias_cc",
                                      tag="bias_cc")
                nc.sync.dma_start(
                    out=bias_cc,
                    in_=bass.AP(tensor=scratch.tensor, offset=scratch.offset
                                + cc * 512, ap=[[0, 128], [1, 512]]))

                # fp16 hi rhs (chs = c*SC), plus scheme-dependent digits
                chs = []
                for h in range(HT):
                    chs_t = csplit.tile([128, 512], F16, name=f"chs{h}",
                                        tag=f"chs{h}")
                    nc.scalar.activation(out=chs_t, in_=cb_f[h],
                                         func=mybir.ActivationFunctionType.Copy,
                                         scale=SC)
                    chs.append(chs_t)
                if use_fp8:
                    cb8 = csplit.tile([128, csplit_blocks, 512], F8,
                                      name="cb8", tag="cb8")
                    for h in range(HT):
                        nc.scalar.copy(out=cb8[:, h, :], in_=cb_f[h])
                    if scheme == "fp8corr2":
                        for h in range(HT):
                            ch16 = strips.tile([128, 512], F16, tag="ch16")
                            nc.scalar.copy(out=ch16, in_=cb_f[h])
                            tmpc = strips.tile([128, 512], F32, tag="tmpc")
                            nc.vector.tensor_sub(out=tmpc, in0=cb_f[h],
                                                 in1=ch16)
                            nc.scalar.activation(
                                out=cb8[:, HT + h, :], in_=tmpc,
                                func=mybir.ActivationFunctionType.Copy,
                                scale=SC)
                    ch = cls = None
                else:
                    cb8 = None
                    ch, cls = [], []
                    for h in range(HT):
                        ch_t = csplit.tile([128, 512], F16, name=f"ch{h}",
                                           tag=f"ch{h}")
                        nc.scalar.copy(out=ch_t, in_=cb_f[h])
                        ch.append(ch_t)
                        if passes == 3:
                            tmpc = strips.tile([128, 512], F32, tag="tmpc")
                            nc.vector.tensor_sub(out=tmpc, in0=cb_f[h],
                                                 in1=ch_t)
                            cls_t = csplit.tile([128, 512], F16,
                                                name=f"cls{h}", tag=f"cls{h}")
                            nc.scalar.activation(
                                out=cls_t, in_=tmpc,
                                func=mybir.ActivationFunctionType.Copy,
                                scale=SC)
                            cls.append(cls_t)

                for t in range(TT):
                    tsl = slice(t * 128, (t + 1) * 128)
                    acc = psB.tile([128, 512], F32, tag="acc")
                    if use_fp8:
                        ndr = csplit_blocks // 2
                        for h in range(HT):
                            nc.tensor.matmul(acc, lhsT=th[h][:, tsl],
                                             rhs=chs[h], start=(h == 0),
                                             stop=False)
                        for p in range(ndr):
                            nc.tensor.matmul(
                                acc, lhsT=t8[:, 2 * p:2 * p + 2, tsl],
                                rhs=cb8[:, 2 * p:2 * p + 2, :],
                                start=False, stop=(p == ndr - 1),
                                perf_mode=DR)
                    elif passes == 3:
                        for h in range(HT):
                            nc.tensor.matmul(acc, lhsT=th[h][:, tsl],
                                             rhs=chs[h], start=(h == 0),
                                             stop=False)
                            nc.tensor.matmul(acc, lhsT=th[h][:, tsl],
                                             rhs=cls[h], start=False,
                                             stop=False)
                        for h in range(HT):
                            nc.tensor.matmul(acc, lhsT=tl[h][:, tsl],
                                             rhs=ch[h], start=False,
                                             stop=(h == HT - 1))
                    else:
                        for h in range(HT):
                            nc.tensor.matmul(acc, lhsT=th[h][:, tsl],
                                             rhs=chs[h], start=(h == 0),
                                             stop=False)
                        for h in range(HT):
                            nc.tensor.matmul(acc, lhsT=tl[h][:, tsl],
                                             rhs=ch[h], start=False,
                                             stop=(h == HT - 1))
                    s = strips.tile([128, 512], F32, tag="s", bufs=8)
                    nc.vector.tensor_sub(out=s, in0=acc, in1=bias_cc)
                    mx8 = strips.tile([128, 8], F32, tag="mx8", bufs=6)
                    nc.vector.max(out=mx8, in_=s)
                    ix8 = strips.tile([128, 8], U32, tag="ix8", bufs=6)
                    nc.vector.max_index(out=ix8, in_max=mx8, in_values=s)
                    nc.gpsimd.tensor_copy(out=cval_all[:, t, cc:cc + 1],
                                          in_=mx8[:, 0:1])
                    nc.gpsimd.tensor_copy(out=cidx_all[:, t, cc:cc + 1],
                                          in_=ix8[:, 0:1])

        # ---------- phase C: combine winners, vectorized over all tiles ---
        # winner = min over chunks of (global_idx + 16384*[cval < rowmax])
        with tc.tile_pool(name=f"fin{rep}", bufs=1) as fin:
            gmx = fin.tile([128, TT], F32)
            nc.vector.tensor_reduce(out=gmx, in_=cval_all,
                                    axis=mybir.AxisListType.X,
                                    op=mybir.AluOpType.max)
            pen = fin.tile([128, TT, CCH], F32)
            nc.vector.tensor_tensor(
                out=pen, in0=cval_all,
                in1=gmx[:, :, None].broadcast_to([128, TT, CCH]),
                op=mybir.AluOpType.is_lt)
            cand = fin.tile([128, TT, CCH], F32)
            nc.vector.tensor_scalar(
                out=cand, in0=pen, scalar1=16384.0, scalar2=None,
                op0=mybir.AluOpType.mult)
            nc.vector.tensor_add(cand, cand, cidx_all)
            nc.vector.tensor_tensor(
                out=cand, in0=cand,
                in1=chunk_off[:, None, :].broadcast_to([128, TT, CCH]),
                op=mybir.AluOpType.add)
            win = fin.tile([128, TT], F32)
            nc.vector.tensor_reduce(out=win, in_=cand,
                                    axis=mybir.AxisListType.X,
                                    op=mybir.AluOpType.min)
            lab = fin.tile([128, TT], I32)
            nc.vector.tensor_copy(lab, win)
            nc.sync.dma_start(out=d_lab[:, :], in_=lab)


def build_nc(scheme=SCHEME, repeat=1):
    nc = bacc.Bacc("TRN2", target_bir_lowering=False, debug=False)

    d = dict(
        x=nc.dram_tensor("x", (L, D), F32, kind="ExternalInput"),
        pw=nc.dram_tensor("pw", (H, D), F32, kind="ExternalInput"),
        lnw=nc.dram_tensor("lnw", (D,), F32, kind="ExternalInput"),
        lnb=nc.dram_tensor("lnb", (D,), F32, kind="ExternalInput"),
        cb=nc.dram_tensor("cb", (H, C), F32, kind="ExternalInput"),
        cbt=nc.dram_tensor("cbt", (C, H), F32, kind="ExternalInput"),
        lab=nc.dram_tensor("labels", (128, TT), I32, kind="ExternalOutput"),
    )

    with tile.TileContext(nc) as tc:
        for rep in range(repeat):
            _emit(nc, tc, d, scheme, rep)

    nc.compile()
    return nc


_NC_CACHE = {}


def make_in_maps(inputs):
    input_values = np.ascontiguousarray(inputs["input_values"], np.float32)
    pw = np.ascontiguousarray(inputs["proj_weight"], np.float32)
    lnw = np.ascontiguousarray(inputs["ln_weight"], np.float32)
    lnb = np.ascontiguousarray(inputs["ln_bias"], np.float32)
    cb = np.ascontiguousarray(inputs["codebook"], np.float32)
    cbt = np.ascontiguousarray(cb.T)

    in_maps = []
    for i in range(N_CORES):
        in_maps.append({
            "x": np.ascontiguousarray(input_values[i]),
            "pw": pw, "lnw": lnw, "lnb": lnb, "cb": cb, "cbt": cbt,
        })
    return in_maps


def kernel(input_values, ln_weight, ln_bias, proj_weight, codebook):
    key = (SCHEME, 1)
    if key not in _NC_CACHE:
        _NC_CACHE[key] = build_nc(SCHEME, 1)
    nc = _NC_CACHE[key]

    in_maps = make_in_maps(dict(
        input_values=input_values, ln_weight=ln_weight, ln_bias=ln_bias,
        proj_weight=proj_weight, codebook=codebook))
    res = run_bass_kernel_spmd(nc, in_maps, core_ids=list(range(N_CORES)))
    out = np.empty((B, L), np.int32)
    for i in range(N_CORES):
        out[i] = res.results[i]["labels"].T.reshape(L)
    return out


# revision 23
# speedup vs baseline: 3.0531x; 1.0382x over previous
"""Trainium2 Bass kernel for BestRQ vector-quantization codebook lookup.

Pipeline (per NeuronCore, data-parallel over batch):
  x (2048,512) --LayerNorm--> xn --PE transpose--> xnT (d-major)
  t^T = projW^T @ xn^T  (fp32 matmul, accumulated over d)
  t split into fp16 hi (th16) and fp8-e4m3 digits for corrections.
  codebook streamed in 512-column chunks as fp16 hi (chs = c*2048) and
  fp8 digits.
  score*2048 accumulates in one PSUM group per (chunk, token-tile):
    main:      th16 @ chs16                  (8 fp16 matmuls, h-blocks)
    t-corr:    tl8 @ ch8                     (4 fp8 DoubleRow matmuls)
    c-corr:    th8 @ cls8  [scheme fp8corr2] (4 fp8 DoubleRow matmuls)
  s = score - 1024*||c||^2 ; per-chunk argmax via DVE max8/max_index;
  vectorized global combine over the 16 chunks at the end.

Schemes (host-emulated label flips / rel-err vs the fp32 reference):
  fp16x3   3 fp16 passes (original baseline)   1 flip   8.8e-4
  fp8corr2 fp16 main + both fp8 DR corrections 2 flips  9.7e-4
  fp8corr  fp16 main + t-side fp8 DR corr      5 flips  8.4e-3
  fp16x2   2 fp16 passes                       5 flips  8.4e-3
"""

import numpy as np

import concourse.bacc as bacc
import concourse.bass as bass
import concourse.mybir as mybir
import concourse.tile as tile
from concourse.bass_utils import run_bass_kernel_spmd
from concourse.masks import make_identity

B, L, D, H, C = 8, 2048, 512, 1024, 8192
LN_EPS = 1e-5
N_CORES = 8

TT = L // 128      # 16 token tiles
CCH = C // 512     # 16 codebook chunks
HT = H // 128      # 8 h tiles
DT = D // 128      # 4 d tiles
TOKC = L // 512    # 4 token chunks (projection)
SC = 2048.0        # 2^11 lo-part scale

F32 = mybir.dt.float32
F16 = mybir.dt.float16
F8 = mybir.dt.float8e4
I32 = mybir.dt.int32
U32 = mybir.dt.uint32

SCHEME = "fp8corr"


def _emit(nc, tc, d, scheme, rep):
    """Emit one full pipeline iteration into the TileContext."""
    DR = mybir.MatmulPerfMode.DoubleRow
    d_x, d_pw, d_lnw, d_lnb, d_cb, d_cbt, d_lab = (
        d["x"], d["pw"], d["lnw"], d["lnb"], d["cb"], d["cbt"], d["lab"])
    ablate_A = scheme.endswith("_A")
    ablate_M = scheme.endswith("_M")
    if ablate_A or ablate_M:
        scheme = scheme[:-2]
    use_fp8 = scheme in ("fp8corr", "fp8corr2")
    csplit_blocks = 16 if scheme == "fp8corr2" else 8
    passes = {"fp16x3": 3, "fp16x2": 2}.get(scheme, 0)

    with tc.tile_pool(name=f"consts{rep}", bufs=1) as consts, \
         tc.tile_pool(name=f"persist{rep}", bufs=1) as persist, \
         tc.tile_pool(name=f"dram{rep}", bufs=1, space="DRAM") as dram:
        scratch = dram.tile([C], F32)

        # ---------- constants ----------
        ident = consts.tile([128, 128], F32)
        make_identity(nc, ident)
        eps_t = consts.tile([128, 1], F32)
        nc.vector.memset(eps_t, LN_EPS)
        lnw_bc = consts.tile([128, D], F32)
        nc.sync.dma_start(
            out=lnw_bc,
            in_=bass.AP(tensor=d_lnw, offset=0, ap=[[0, 128], [1, D]]))
        lnb_bc = consts.tile([128, D], F32)
        nc.sync.dma_start(
            out=lnb_bc,
            in_=bass.AP(tensor=d_lnb, offset=0, ap=[[0, 128], [1, D]]))
        chunk_off = consts.tile([128, CCH], F32)
        for j in range(CCH):
            nc.vector.memset(chunk_off[:, j:j + 1], 512.0 * j)

        # persistent t^T tiles: fp16 hi + correction digits
        th = [persist.tile([128, L], F16, name=f"th{h}_{rep}", tag=f"th{h}")
              for h in range(HT)]
        if use_fp8:
            t8 = persist.tile([128, csplit_blocks, L], F8, name=f"t8_{rep}",
                              tag="t8")
            tl = None
        else:
            t8 = None
            tl = [persist.tile([128, L], F16, name=f"tl{h}_{rep}",
                               tag=f"tl{h}") for h in range(HT)]

        # ---------- phase A: LN + transposes + projection + split ----------
        # projection scheme (fp8 variants): fp16 main + fp8-DR corrections.
        #   t = xnh16 @ pwh16  +  2^-17 * (xnl8 @ pwh8[*64] + xnh8 @ pwl8)
        # where xnl8 = fp8((xn - xnh16)*SC), pwh8 = fp8(pwh*64),
        #       pwl8 = fp8((pw - pwh16)*SC*64), xnh8 = fp8(xn).
        # Both corrections arrive at scale 64*SC = 2^17 and accumulate in
        # one PSUM group; |t| error ~2^-15 (negligible vs the cross scheme).
        with tc.tile_pool(name=f"phA{rep}", bufs=1) as phA, \
             tc.tile_pool(name=f"ldtmp{rep}", bufs=3) as ldtmp:
            if use_fp8:
                pwh16 = [phA.tile([128, H], F16, name=f"pwh16_{dd}_{rep}",
                                  tag=f"pwh16_{dd}") for dd in range(DT)]
                pw8 = phA.tile([128, 2 * DT, H], F8, name=f"pw8_{rep}",
                               tag="pw8")
                xnh16 = [phA.tile([128, L], F16, name=f"xnh16_{dd}_{rep}",
                                  tag=f"xnh16_{dd}") for dd in range(DT)]
                xn8 = phA.tile([128, 2 * DT, L], F8, name=f"xn8_{rep}",
                               tag="xn8")
                pwT = xnT = None
            else:
                pwT = [phA.tile([128, H], F32, name=f"pwT{dd}_{rep}",
                                tag=f"pwT{dd}") for dd in range(DT)]
                xnT = [phA.tile([128, L], F32, name=f"xnT{dd}_{rep}",
                                tag=f"xnT{dd}") for dd in range(DT)]

            def emit_proj_tk(tk, pool):
                """Emit projection groups for one 512-token chunk (fp8 path)."""
                for dd in range(DT):
                    nc.scalar.copy(
                        out=xn8[:, DT + dd, tk * 512:(tk + 1) * 512],
                        in_=xnh16[dd][:, tk * 512:(tk + 1) * 512])
                for h in range(HT):
                    tsl = slice(tk * 512, (tk + 1) * 512)
                    hsl = slice(h * 128, (h + 1) * 128)
                    ps_t = pool.tile([128, 512], F32, tag="ps_t", bufs=3)
                    for dd in range(DT):
                        nc.tensor.matmul(
                            ps_t, lhsT=pwh16[dd][:, hsl],
                            rhs=xnh16[dd][:, tsl],
                            start=(dd == 0), stop=(dd == DT - 1))
                    ps_c = pool.tile([128, 512], F32, tag="ps_c", bufs=3)
                    for p in range(DT):
                        nc.tensor.matmul(
                            ps_c,
                            lhsT=pw8[:, 2 * p:2 * p + 2, hsl],
                            rhs=xn8[:, 2 * p:2 * p + 2, tsl],
                            start=(p == 0), stop=(p == DT - 1),
                            perf_mode=DR)
                    corr = ldtmp.tile([128, 512], F32, tag="corr")
                    nc.scalar.activation(
                        out=corr, in_=ps_c,
                        func=mybir.ActivationFunctionType.Copy,
                        scale=1.0 / (64.0 * SC))
                    t_f = ldtmp.tile([128, 512], F32, tag="t_f")
                    nc.vector.tensor_add(out=t_f, in0=ps_t, in1=corr)
                    nc.scalar.copy(out=th[h][:, tsl], in_=t_f)
                    tmp = ldtmp.tile([128, 512], F32, tag="split_tmp")
                    nc.vector.tensor_sub(out=tmp, in0=t_f, in1=th[h][:, tsl])
                    nc.scalar.activation(
                        out=t8[:, h, tsl], in_=tmp,
                        func=mybir.ActivationFunctionType.Copy, scale=SC)
                    if scheme == "fp8corr2":
                        nc.scalar.copy(out=t8[:, HT + h, tsl],
                                       in_=th[h][:, tsl])

            with tc.tile_pool(name=f"psTr{rep}", bufs=(2 if use_fp8 else 4),
                              space="PSUM") as psTr:
                # proj weight: load (h,d), PE-transpose to (d,h); split
                # straight off the transpose PSUM (fp8 path keeps no fp32
                # d-major copy; the fp8-hi casts happen in bulk later).
                for h in range(HT):
                    pw_t = ldtmp.tile([128, D], F32, tag="pw_t")
                    nc.sync.dma_start(out=pw_t,
                                      in_=d_pw[h * 128:(h + 1) * 128, :])
                    for dd in range(DT):
                        ps_tr = psTr.tile([128, 128], F32, tag="ps_tr")
                        nc.tensor.transpose(
                            ps_tr, pw_t[:, dd * 128:(dd + 1) * 128], ident)
                        hsl = slice(h * 128, (h + 1) * 128)
                        if use_fp8:
                            nc.scalar.copy(out=pwh16[dd][:, hsl], in_=ps_tr)
                            tmpw = ldtmp.tile([128, 128], F32, tag="tmpw")
                            nc.vector.tensor_sub(out=tmpw, in0=ps_tr,
                                                 in1=pwh16[dd][:, hsl])
                            nc.scalar.activation(
                                out=pw8[:, DT + dd, hsl], in_=tmpw,
                                func=mybir.ActivationFunctionType.Copy,
                                scale=SC * 64.0)
                        else:
                            nc.scalar.copy(out=pwT[dd][:, hsl], in_=ps_tr)
                if use_fp8:
                    for dd in range(DT):
                        nc.scalar.activation(
                            out=pw8[:, dd, :], in_=pwh16[dd],
                            func=mybir.ActivationFunctionType.Copy, scale=64.0)

                # LayerNorm + transpose to d-major; split off transpose PSUM
                for t in range(TT):
                    x_t = ldtmp.tile([128, D], F32, tag="x_t", bufs=6)
                    nc.sync.dma_start(out=x_t,
                                      in_=d_x[t * 128:(t + 1) * 128, :])
                    stats = ldtmp.tile([128, 6], F32, tag="stats")
                    nc.vector.bn_stats(out=stats, in_=x_t)
                    mv = ldtmp.tile([128, 2], F32, tag="mv")
                    nc.vector.bn_aggr(out=mv, in_=stats)
                    rstd = ldtmp.tile([128, 1], F32, tag="rstd")
                    nc.scalar.activation(out=rstd, in_=mv[:, 1:2],
                                         func=mybir.ActivationFunctionType.Sqrt,
                                         bias=eps_t, scale=1.0)
                    nc.vector.reciprocal(out=rstd, in_=rstd)
                    xn = ldtmp.tile([128, D], F32, tag="xn")
                    nc.vector.tensor_scalar(
                        out=xn, in0=x_t, scalar1=mv[:, 0:1], scalar2=rstd,
                        op0=mybir.AluOpType.subtract, op1=mybir.AluOpType.mult)
                    nc.vector.tensor_mul(out=xn, in0=xn, in1=lnw_bc)
                    nc.vector.tensor_add(out=xn, in0=xn, in1=lnb_bc)
                    for dd in range(DT):
                        ps_tr = psTr.tile([128, 128], F32, tag="ps_tr")
                        nc.tensor.transpose(
                            ps_tr, xn[:, dd * 128:(dd + 1) * 128], ident)
                        tsl128 = slice(t * 128, (t + 1) * 128)
                        if use_fp8:
                            nc.scalar.copy(out=xnh16[dd][:, tsl128],
                                           in_=ps_tr)
                            tmpx = ldtmp.tile([128, 128], F32, tag="tmpx")
                            nc.vector.tensor_sub(out=tmpx, in0=ps_tr,
                                                 in1=xnh16[dd][:, tsl128])
                            nc.scalar.activation(
                                out=xn8[:, dd, tsl128], in_=tmpx,
                                func=mybir.ActivationFunctionType.Copy,
                                scale=SC)
                        else:
                            nc.scalar.copy(out=xnT[dd][:, tsl128], in_=ps_tr)
                    if use_fp8 and t % 4 == 3:
                        emit_proj_tk(t // 4, psTr)

            # non-fp8 projection (fp8 path emits proj inline above)
            if not use_fp8:
                with tc.tile_pool(name=f"psA{rep}", bufs=2,
                                  space="PSUM") as psA:
                    for tk in range(TOKC):
                        for h in range(HT):
                            tsl = slice(tk * 512, (tk + 1) * 512)
                            hsl = slice(h * 128, (h + 1) * 128)
                            ps_t = psA.tile([128, 512], F32, tag="ps_t",
                                            bufs=4)
                            for dd in range(DT):
                                nc.tensor.matmul(
                                    ps_t,
                                    lhsT=pwT[dd][:, hsl],
                                    rhs=xnT[dd][:, tsl],
                                    start=(dd == 0), stop=(dd == DT - 1))
                            nc.scalar.copy(out=th[h][:, tsl], in_=ps_t)
                            tmp = ldtmp.tile([128, 512], F32, tag="split_tmp")
                            nc.vector.tensor_sub(out=tmp, in0=ps_t,
                                                 in1=th[h][:, tsl])
                            nc.scalar.activation(
                                out=tl[h][:, tsl], in_=tmp,
                                func=mybir.ActivationFunctionType.Copy,
                                scale=SC)

        # ---------- phase B: cross matmul + per-chunk argmax ----------
        if ablate_A:
            # phase-A timing ablation: emit a trivial output write and stop
            with tc.tile_pool(name=f"abl{rep}", bufs=1) as abl:
                z = abl.tile([128, TT], I32)
                nc.vector.tensor_copy(z, th[0][:, 0:TT])
                nc.sync.dma_start(out=d_lab[:, :], in_=z)
            return
        cval_all = persist.tile([128, TT, CCH], F32, name=f"cval_{rep}",
                                tag="cval")
        cidx_all = persist.tile([128, TT, CCH], F32, name=f"cidx_{rep}",
                                tag="cidx")

        with tc.tile_pool(name=f"cbf{rep}", bufs=1) as cbf_pool, \
             tc.tile_pool(name=f"csplit{rep}", bufs=3) as csplit, \
             tc.tile_pool(name=f"strips{rep}", bufs=4) as strips, \
             tc.tile_pool(name=f"psB{rep}", bufs=8, space="PSUM") as psB:

            for cc in range(CCH):
                csl = slice(cc * 512, (cc + 1) * 512)
                cb_f = []
                for h in range(HT):
                    t_ = cbf_pool.tile([128, 512], F32, name=f"cbf{h}",
                                       tag=f"cbf{h}")
                    nc.sync.dma_start(out=t_, in_=d_cb[h * 128:(h + 1) * 128,
                                                       csl])
                    cb_f.append(t_)
                # bias_cc = 1024 * sum_h c^2 for this chunk's codewords:
                # square + free-dim reduce over cbT rows (c on partitions),
                # then a DRAM bounce to re-layout as (128 bcast, 512 c).
                csq_cols = csplit.tile([128, 4], F32, name="csq_cols",
                                       tag="csq_cols")
                for j in range(4):
                    cbt_t = csplit.tile([128, H], F32, name="cbt_t",
                                        tag="cbt_t", bufs=3)
                    nc.sync.dma_start(
                        out=cbt_t,
                        in_=d_cbt[cc * 512 + j * 128:cc * 512 + (j + 1) * 128, :])
                    sq_t = csplit.tile([128, H], F32, name="sq_t",
                                       tag="sq_t", bufs=3)
                    nc.scalar.activation(out=sq_t, in_=cbt_t,
                                         func=mybir.ActivationFunctionType.Square)
                    nc.vector.tensor_reduce(
                        out=csq_cols[:, j:j + 1], in_=sq_t,
                        axis=mybir.AxisListType.X, op=mybir.AluOpType.add)
                nc.vector.tensor_scalar_mul(csq_cols, csq_cols, SC * 0.5)
                nc.sync.dma_start(
                    out=bass.AP(tensor=scratch.tensor, offset=scratch.offset
                                + cc * 512, ap=[[1, 128], [128, 4]]),
                    in_=csq_cols)
                bias_cc = csplit.tile([128, 512], F32, name="bias_cc",
                                      tag="bias_cc")
                nc.sync.dma_start(
                    out=bias_cc,
                    in_=bass.AP(tensor=scratch.tensor, offset=scratch.offset
                                + cc * 512, ap=[[0, 128], [1, 512]]))

                # fp16 hi rhs (chs = c*SC), plus scheme-dependent digits
                chs = []
                for h in range(HT):
                    chs_t = csplit.tile([128, 512], F16, name=f"chs{h}",
                                        tag=f"chs{h}")
                    nc.scalar.activation(out=chs_t, in_=cb_f[h],
                                         func=mybir.ActivationFunctionType.Copy,
                                         scale=SC)
                    chs.append(chs_t)
                if use_fp8:
                    cb8 = csplit.tile([128, csplit_blocks, 512], F8,
                                      name="cb8", tag="cb8")
                    for h in range(HT):
                        nc.scalar.copy(out=cb8[:, h, :], in_=cb_f[h])
                    if scheme == "fp8corr2":
                        for h in range(HT):
                            ch16 = strips.tile([128, 512], F16, tag="ch16")
                            nc.scalar.copy(out=ch16, in_=cb_f[h])
                            tmpc = strips.tile([128, 512], F32, tag="tmpc")
                            nc.vector.tensor_sub(out=tmpc, in0=cb_f[h],
                                                 in1=ch16)
                            nc.scalar.activation(
                                out=cb8[:, HT + h, :], in_=tmpc,
                                func=mybir.ActivationFunctionType.Copy,
                                scale=SC)
                    ch = cls = None
                else:
                    cb8 = None
                    ch, cls = [], []
                    for h in range(HT):
                        ch_t = csplit.tile([128, 512], F16, name=f"ch{h}",
                                           tag=f"ch{h}")
                        nc.scalar.copy(out=ch_t, in_=cb_f[h])
                        ch.append(ch_t)
                        if passes == 3:
                            tmpc = strips.tile([128, 512], F32, tag="tmpc")
                            nc.vector.tensor_sub(out=tmpc, in0=cb_f[h],
                                                 in1=ch_t)
                            cls_t = csplit.tile([128, 512], F16,
                                                name=f"cls{h}", tag=f"cls{h}")
                            nc.scalar.activation(
                                out=cls_t, in_=tmpc,
                                func=mybir.ActivationFunctionType.Copy,
                                scale=SC)
                            cls.append(cls_t)

                for t in range(TT):
                    tsl = slice(t * 128, (t + 1) * 128)
                    acc = psB.tile([128, 512], F32, tag="acc")
                    if use_fp8:
                        ndr = csplit_blocks // 2
                        for h in range(HT):
                            nc.tensor.matmul(acc, lhsT=th[h][:, tsl],
                                             rhs=chs[h], start=(h == 0),
                                             stop=False)
                        for p in range(ndr):
                            nc.tensor.matmul(
                                acc, lhsT=t8[:, 2 * p:2 * p + 2, tsl],
                                rhs=cb8[:, 2 * p:2 * p + 2, :],
                                start=False, stop=(p == ndr - 1),
                                perf_mode=DR)
                    elif passes == 3:
                        for h in range(HT):
                            nc.tensor.matmul(acc, lhsT=th[h][:, tsl],
                                             rhs=chs[h], start=(h == 0),
                                             stop=False)
                            nc.tensor.matmul(acc, lhsT=th[h][:, tsl],
                                             rhs=cls[h], start=False,
                                             stop=False)
                        for h in range(HT):
                            nc.tensor.matmul(acc, lhsT=tl[h][:, tsl],
                                             rhs=ch[h], start=False,
                                             stop=(h == HT - 1))
                    else:
                        for h in range(HT):
                            nc.tensor.matmul(acc, lhsT=th[h][:, tsl],
                                             rhs=chs[h], start=(h == 0),
                                             stop=False)
                        for h in range(HT):
                            nc.tensor.matmul(acc, lhsT=tl[h][:, tsl],
                                             rhs=ch[h], start=False,
                                             stop=(h == HT - 1))
                    if ablate_M:
                        # matmul-floor ablation: minimal PSUM drain on ACT
                        mn = strips.tile([128, 1], F32, tag="mn", bufs=8)
                        nc.scalar.copy(out=mn, in_=acc[:, 0:1])
                        nc.gpsimd.tensor_copy(out=cval_all[:, t, cc:cc + 1],
                                              in_=mn)
                        nc.gpsimd.tensor_copy(out=cidx_all[:, t, cc:cc + 1],
                                              in_=mn)
                        continue
                    s = strips.tile([128, 512], F32, tag="s", bufs=8)
                    nc.vector.tensor_sub(out=s, in0=acc, in1=bias_cc)
                    mx8 = strips.tile([128, 8], F32, tag="mx8", bufs=6)
                    nc.vector.max(out=mx8, in_=s)
                    ix8 = strips.tile([128, 8], U32, tag="ix8", bufs=6)
                    nc.vector.max_index(out=ix8, in_max=mx8, in_values=s)
                    nc.gpsimd.tensor_copy(out=cval_all[:, t, cc:cc + 1],
                                          in_=mx8[:, 0:1])
                    nc.gpsimd.tensor_copy(out=cidx_all[:, t, cc:cc + 1],
                                          in_=ix8[:, 0:1])

        # ---------- phase C: combine winners, vectorized over all tiles ---
        # winner = min over chunks of (global_idx + 16384*[cval < rowmax])
        with tc.tile_pool(name=f"fin{rep}", bufs=1) as fin:
            gmx = fin.tile([128, TT], F32)
            nc.vector.tensor_reduce(out=gmx, in_=cval_all,
                                    axis=mybir.AxisListType.X,
                                    op=mybir.AluOpType.max)
            pen = fin.tile([128, TT, CCH], F32)
            nc.vector.tensor_tensor(
                out=pen, in0=cval_all,
                in1=gmx[:, :, None].broadcast_to([128, TT, CCH]),
                op=mybir.AluOpType.is_lt)
            cand = fin.tile([128, TT, CCH], F32)
            nc.vector.tensor_scalar(
                out=cand, in0=pen, scalar1=16384.0, scalar2=None,
                op0=mybir.AluOpType.mult)
            nc.vector.tensor_add(cand, cand, cidx_all)
            nc.vector.tensor_tensor(
                out=cand, in0=cand,
                in1=chunk_off[:, None, :].broadcast_to([128, TT, CCH]),
                op=mybir.AluOpType.add)
            win = fin.tile([128, TT], F32)
            nc.vector.tensor_reduce(out=win, in_=cand,
                                    axis=mybir.AxisListType.X,
                                    op=mybir.AluOpType.min)
            lab = fin.tile([128, TT], I32)
            nc.vector.tensor_copy(lab, win)
            nc.sync.dma_start(out=d_lab[:, :], in_=lab)


def build_nc(scheme=SCHEME, repeat=1):
    nc = bacc.Bacc("TRN2", target_bir_lowering=False, debug=False)

    d = dict(
        x=nc.dram_tensor("x", (L, D), F32, kind="ExternalInput"),
        pw=nc.dram_tensor("pw", (H, D), F32, kind="ExternalInput"),
        lnw=nc.dram_tensor("lnw", (D,), F32, kind="ExternalInput"),
        lnb=nc.dram_tensor("lnb", (D,), F32, kind="ExternalInput"),
        cb=nc.dram_tensor("cb", (H, C), F32, kind="ExternalInput"),
        cbt=nc.dram_tensor("cbt", (C, H), F32, kind="ExternalInput"),
        lab=nc.dram_tensor("labels", (128, TT), I32, kind="ExternalOutput"),
    )

    with tile.TileContext(nc) as tc:
        for rep in range(repeat):
            _emit(nc, tc, d, scheme, rep)

    nc.compile()
    return nc


_NC_CACHE = {}


def make_in_maps(inputs):
    input_values = np.ascontiguousarray(inputs["input_values"], np.float32)
    pw = np.ascontiguousarray(inputs["proj_weight"], np.float32)
    lnw = np.ascontiguousarray(inputs["ln_weight"], np.float32)
    lnb = np.ascontiguousarray(inputs["ln_bias"], np.float32)
    cb = np.ascontiguousarray(inputs["codebook"], np.float32)
    cbt = np.ascontiguousarray(cb.T)

    in_maps = []
    for i in range(N_CORES):
        in_maps.append({
            "x": np.ascontiguousarray(input_values[i]),
            "pw": pw, "lnw": lnw, "lnb": lnb, "cb": cb, "cbt": cbt,
        })
    return in_maps


def kernel(input_values, ln_weight, ln_bias, proj_weight, codebook):
    key = (SCHEME, 1)
    if key not in _NC_CACHE:
        _NC_CACHE[key] = build_nc(SCHEME, 1)
    nc = _NC_CACHE[key]

    in_maps = make_in_maps(dict(
        input_values=input_values, ln_weight=ln_weight, ln_bias=ln_bias,
        proj_weight=proj_weight, codebook=codebook))
    res = run_bass_kernel_spmd(nc, in_maps, core_ids=list(range(N_CORES)))
    out = np.empty((B, L), np.int32)
    for i in range(N_CORES):
        out[i] = res.results[i]["labels"].T.reshape(L)
    return out
